# revision 2
# baseline (speedup 1.0000x reference)
"""MoE layer (top-2 of 8 experts, d=1024, d_ff=4096) on 8 TRN2 NeuronCores.

Strategy: expert-parallel. The router / top-k / softmax-gate computation is
tiny (0.05% of FLOPs) and runs on host in numpy; tokens are then grouped by
expert and dispatched so that core e runs the full FFN (x @ W1[e], swish,
@ W2[e], gate scale) for exactly the tokens routed to expert e, padded to a
common capacity C. All matmuls run in bf16 with fp32 PSUM accumulation.
Host gathers the per-(token,slot) rows and adds the two expert contributions
per token.
"""

import math

import numpy as np
import ml_dtypes

D_MODEL, D_FF, N_EXPERTS, TOP_K = 1024, 4096, 8, 2
N_CORES = 8
P = 128
TN = 512  # token tile (free dim of the first matmul)

_KERNEL_CACHE: dict[int, object] = {}


def _build_device_kernel(C: int):
    """Per-core program: y[C,1024] = gate * (swish(x @ W1) @ W2) for C tokens.

    Inputs (per core): xT [1024, C] bf16 (token activations, d-major),
    w1 [1024, 4096] bf16, w2 [4096, 1024] bf16, g [C] f32 (gate weights,
    0 for padding rows). Output y [C, 1024] f32.
    """
    import concourse.bass as bass  # noqa: F401
    import concourse.mybir as mybir
    import concourse.tile as tile
    from concourse import bacc

    dt = mybir.dt
    KO = D_MODEL // P  # 8 contraction tiles for matmul 1
    MF = D_FF // P  # 32 dff tiles
    NT = C // P  # token 128-blocks
    assert C % P == 0

    nc = bacc.Bacc("TRN2", target_bir_lowering=False, debug=False)

    xT = nc.dram_tensor("xT", [D_MODEL, C], dt.bfloat16, kind="ExternalInput")
    w1 = nc.dram_tensor("w1", [D_MODEL, D_FF], dt.bfloat16, kind="ExternalInput")
    w2 = nc.dram_tensor("w2", [D_FF, D_MODEL], dt.bfloat16, kind="ExternalInput")
    g = nc.dram_tensor("g", [C], dt.float32, kind="ExternalInput")
    y = nc.dram_tensor("y", [C, D_MODEL], dt.float32, kind="ExternalOutput")

    xT_r = xT.ap().rearrange("(ko p) n -> p ko n", p=P)
    w1_r = w1.ap().rearrange("(ko p) f -> p ko f", p=P)
    w2_r = w2.ap().rearrange("(mf p) f -> p mf f", p=P)
    g_r = g.ap().rearrange("(a p) -> p a", p=P)
    y_r = y.ap().rearrange("(a p) n -> p a n", p=P)

    # token tiles: full TN tiles plus remainder (multiple of 128)
    tiles = []
    t0 = 0
    while t0 < C:
        tn = min(TN, C - t0)
        tiles.append((t0, tn))
        t0 += tn

    with tile.TileContext(nc) as tc:
        with (
            tc.tile_pool(name="wpool", bufs=1) as wpool,
            tc.tile_pool(name="gpool", bufs=1) as gpool,
            tc.tile_pool(name="xpool", bufs=2) as xpool,
            tc.tile_pool(name="hpool", bufs=1) as hpool,
            tc.tile_pool(name="ypool", bufs=3) as ypool,
            tc.tile_pool(name="ps1", bufs=3, space="PSUM") as ps1,
            tc.tile_pool(name="ps2", bufs=3, space="PSUM") as ps2,
        ):
            w1_sb = wpool.tile([P, KO, D_FF], dt.bfloat16)
            w2_sb = wpool.tile([P, MF, D_MODEL], dt.bfloat16)
            g_sb = gpool.tile([P, NT], dt.float32)

            nc.sync.dma_start(g_sb[:], g_r)
            # chunk weight loads so early matmuls can start before the full
            # matrices land
            W1_CHUNK = D_FF // 8
            for j in range(8):
                s = slice(j * W1_CHUNK, (j + 1) * W1_CHUNK)
                nc.sync.dma_start(w1_sb[:, :, s], w1_r[:, :, s])
            W2_CHUNK = MF // 8
            for j in range(8):
                s = slice(j * W2_CHUNK, (j + 1) * W2_CHUNK)
                nc.sync.dma_start(w2_sb[:, s, :], w2_r[:, s, :])

            for t0, tn in tiles:
                x_sb = xpool.tile([P, KO, TN], dt.bfloat16, tag="x", name="x_sb")
                nc.sync.dma_start(x_sb[:, :, :tn], xT_r[:, :, t0 : t0 + tn])

                # H^T tile [dff, tn] in bf16, dff-major on partitions
                h_sb = hpool.tile([P, MF, TN], dt.bfloat16, tag="h", name="h_sb")

                for mf in range(MF):
                    ph = ps1.tile([P, TN], dt.float32, tag="ph", name="ph")
                    for ko in range(KO):
                        nc.tensor.matmul(
                            ph[:, :tn],
                            w1_sb[:, ko, mf * P : (mf + 1) * P],
                            x_sb[:, ko, :tn],
                            start=(ko == 0),
                            stop=(ko == KO - 1),
                        )
                    nc.scalar.activation(
                        h_sb[:, mf, :tn],
                        ph[:, :tn],
                        mybir.ActivationFunctionType.Silu,
                    )

                for mt in range(tn // P):
                    mtg = t0 // P + mt
                    y_sb = ypool.tile([P, D_MODEL], dt.float32, tag="y", name="y_sb")
                    for nf in range(D_MODEL // 512):
                        py = ps2.tile([P, 512], dt.float32, tag="py", name="py")
                        for mf in range(MF):
                            nc.tensor.matmul(
                                py[:],
                                h_sb[:, mf, mt * P : (mt + 1) * P],
                                w2_sb[:, mf, nf * 512 : (nf + 1) * 512],
                                start=(mf == 0),
                                stop=(mf == MF - 1),
                            )
                        nc.vector.tensor_scalar_mul(
                            y_sb[:, nf * 512 : (nf + 1) * 512],
                            py[:],
                            g_sb[:, mtg : mtg + 1],
                        )
                    nc.sync.dma_start(y_r[:, mtg, :], y_sb[:])

    nc.compile()
    return nc


def _route(xf: np.ndarray, router: np.ndarray):
    """Host-side top-2 routing. Returns per-pair expert assignment grouped by
    expert with padding to capacity C (multiple of 128)."""
    T = xf.shape[0]
    logits = xf @ router  # [T, E] f32
    # top-2 (desc value, ties -> lower index, matching jax.lax.top_k)
    ti = np.argsort(-logits, axis=1, kind="stable")[:, :TOP_K]  # [T, 2]
    tv = np.take_along_axis(logits, ti, axis=1)
    e = np.exp(tv - tv[:, 0:1])
    w = (e / e.sum(axis=1, keepdims=True)).astype(np.float32)  # [T, 2]

    experts_all = ti.T.ravel()  # [2T] slot-major
    gates_all = w.T.ravel()
    tokens_all = np.tile(np.arange(T, dtype=np.int64), TOP_K)

    order = np.argsort(experts_all, kind="stable")
    sorted_experts = experts_all[order]
    counts = np.bincount(sorted_experts, minlength=N_EXPERTS)
    C = max(512, int(math.ceil(counts.max() / P)) * P)
    starts = np.concatenate([[0], np.cumsum(counts)[:-1]])
    rank = np.arange(TOP_K * T) - starts[sorted_experts]
    slot_of_pair = np.empty(TOP_K * T, dtype=np.int64)
    slot_of_pair[order] = sorted_experts * C + rank

    slot_token = np.full(N_EXPERTS * C, T, dtype=np.int64)  # T = zero sentinel
    slot_token[slot_of_pair] = tokens_all
    slot_gate = np.zeros(N_EXPERTS * C, dtype=np.float32)
    slot_gate[slot_of_pair] = gates_all
    return C, slot_token, slot_gate, slot_of_pair


def kernel(x, router, W1, W2, _trace=False):
    from concourse.bass_utils import run_bass_kernel_spmd

    B, S, d = x.shape
    T = B * S
    xf = np.ascontiguousarray(x.reshape(T, d), dtype=np.float32)

    C, slot_token, slot_gate, slot_of_pair = _route(xf, np.asarray(router))

    bf16 = ml_dtypes.bfloat16
    # [d, T+1] bf16 with a trailing zero column as padding sentinel
    xfT = np.concatenate(
        [xf.T, np.zeros((d, 1), np.float32)], axis=1
    ).astype(bf16)
    xT_all = xfT[:, slot_token]  # [d, 8C]
    W1b = np.asarray(W1).astype(bf16)
    W2b = np.asarray(W2).astype(bf16)

    nc = _KERNEL_CACHE.get(C)
    if nc is None:
        nc = _build_device_kernel(C)
        _KERNEL_CACHE[C] = nc

    in_maps = []
    for e in range(N_EXPERTS):
        in_maps.append(
            {
                "xT": np.ascontiguousarray(xT_all[:, e * C : (e + 1) * C]),
                "w1": np.ascontiguousarray(W1b[e]),
                "w2": np.ascontiguousarray(W2b[e]),
                "g": np.ascontiguousarray(slot_gate[e * C : (e + 1) * C]),
            }
        )

    kw = {}
    if _trace:
        kw = {"trace": True, "trace_cores": list(range(N_CORES))}
    res = run_bass_kernel_spmd(nc, in_maps, core_ids=list(range(N_CORES)), **kw)
    y_rows = np.concatenate([res.results[e]["y"] for e in range(N_EXPERTS)], axis=0)

    out = y_rows[slot_of_pair[:T]] + y_rows[slot_of_pair[T:]]
    out = out.reshape(B, S, d).astype(np.float32)
    if _trace:
        return out, res
    return out


# revision 3
# speedup vs baseline: 1.1121x; 1.1121x over previous
"""MoE layer (top-2 of 8 experts, d=1024, d_ff=4096) on 8 TRN2 NeuronCores.

Strategy: d_ff-parallel (tensor-parallel over the FFN hidden dim). The
router / top-k / softmax-gate computation is tiny (0.05% of FLOPs) and runs
on host in numpy. Tokens are grouped by expert into a single padded stream of
R rows (each expert segment padded to a multiple of 128). Every core
processes ALL R rows but only a 512-wide slice of d_ff:

    y_c = gate * (swish(x @ W1[e][:, c*512:(c+1)*512]) @ W2[e][c*512:(c+1)*512, :])

so per-core work is exactly 1/8 of the routed FLOPs with zero load
imbalance. The host sums the 8 partial outputs and combines the two expert
contributions per token. All matmuls run in bf16 with fp32 PSUM
accumulation; partial outputs return as bf16 (summed in f32 on host).

All device inputs are host-prearranged to partition-major layouts so every
DMA is per-partition contiguous.
"""

import math

import numpy as np
import ml_dtypes

D_MODEL, D_FF, N_EXPERTS, TOP_K = 1024, 4096, 8, 2
N_CORES = 8
P = 128
TN = 512  # token tile (free dim of matmul 1)
DS = D_FF // N_CORES  # 512: per-core d_ff slice
KO = D_MODEL // P  # 8 contraction tiles for matmul 1
MS = DS // P  # 4 d_ff subtiles per core

_KERNEL_CACHE: dict[tuple, object] = {}


def _build_device_kernel(seg_sizes: tuple[int, ...]):
    """Per-core program over the padded token stream.

    seg_sizes: per-expert padded token counts (multiples of 128), length 8.
    Inputs (per core):
      x  [128, 8, R]        bf16  x^T, d-major:  x[p, ko, r] = xf[tok(r), ko*128+p]
      w1 [128, 8, 8, 512]   bf16  w1[p, e, ko, f] = W1[e, ko*128+p, c*512+f]
      w2 [128, 8, 4, 1024]  bf16  w2[p, e, m, n] = W2[e, c*512+m*128+p, n]
      g  [128, R//128]      f32   g[p, a] = gate[a*128+p]
    Output:
      y  [128, R//128, 1024] bf16 partial FFN output (gate-scaled)
    """
    import concourse.mybir as mybir
    import concourse.tile as tile
    from concourse import bacc

    dt = mybir.dt
    R = sum(seg_sizes)
    NT = R // P

    nc = bacc.Bacc("TRN2", target_bir_lowering=False, debug=False)

    x = nc.dram_tensor("x", [P, KO, R], dt.bfloat16, kind="ExternalInput")
    w1 = nc.dram_tensor("w1", [P, N_EXPERTS, KO, DS], dt.bfloat16, kind="ExternalInput")
    w2 = nc.dram_tensor("w2", [P, N_EXPERTS, MS, D_MODEL], dt.bfloat16, kind="ExternalInput")
    g = nc.dram_tensor("g", [P, NT], dt.float32, kind="ExternalInput")
    y = nc.dram_tensor("y", [P, NT, D_MODEL], dt.bfloat16, kind="ExternalOutput")

    # token tiles: (expert, start, size), sizes <= TN, multiples of 128,
    # never crossing an expert-segment boundary
    tiles = []
    t0 = 0
    for e, seg in enumerate(seg_sizes):
        off = 0
        while off < seg:
            tn = min(TN, seg - off)
            tiles.append((e, t0 + off, tn))
            off += tn
        t0 += seg

    with tile.TileContext(nc) as tc:
        with (
            tc.tile_pool(name="wpool", bufs=1) as wpool,
            tc.tile_pool(name="gpool", bufs=1) as gpool,
            tc.tile_pool(name="xpool", bufs=3) as xpool,
            tc.tile_pool(name="hpool", bufs=2) as hpool,
            tc.tile_pool(name="ypool", bufs=4) as ypool,
            tc.tile_pool(name="ps1", bufs=3, space="PSUM") as ps1,
            tc.tile_pool(name="ps2", bufs=4, space="PSUM") as ps2,
        ):
            w1_sb = wpool.tile([P, N_EXPERTS, KO, DS], dt.bfloat16)
            w2_sb = wpool.tile([P, N_EXPERTS, MS, D_MODEL], dt.bfloat16)
            g_sb = gpool.tile([P, NT], dt.float32)

            # critical path: first x tile + first needed expert's w1 go first,
            # on the sync HWDGE ring; w2/g stream on the scalar ring.
            first_e = tiles[0][0]
            x_order = sorted(range(len(tiles)), key=lambda i: i)
            expert_order = [first_e] + [e for e in range(N_EXPERTS) if e != first_e]

            x_tiles_sb = {}
            # prefetch the first two x tiles before the weight bulk
            for i in range(min(2, len(tiles))):
                e, t0, tn = tiles[i]
                xt = xpool.tile([P, KO, TN], dt.bfloat16, tag="x", name="x_sb")
                nc.sync.dma_start(xt[:, :, :tn], x.ap()[:, :, t0 : t0 + tn])
                x_tiles_sb[i] = xt

            for e in expert_order:
                nc.sync.dma_start(w1_sb[:, e], w1.ap()[:, e])
            nc.scalar.dma_start(g_sb[:], g.ap()[:])
            for e in expert_order:
                nc.scalar.dma_start(w2_sb[:, e], w2.ap()[:, e])

            for i, (e, t0, tn) in enumerate(tiles):
                if i in x_tiles_sb:
                    x_sb = x_tiles_sb[i]
                else:
                    x_sb = xpool.tile([P, KO, TN], dt.bfloat16, tag="x", name="x_sb")
                    nc.sync.dma_start(x_sb[:, :, :tn], x.ap()[:, :, t0 : t0 + tn])

                # H^T tile [dff_slice, tn] bf16
                h_sb = hpool.tile([P, MS, TN], dt.bfloat16, tag="h", name="h_sb")
                for mf in range(MS):
                    ph = ps1.tile([P, TN], dt.float32, tag="ph", name="ph")
                    for ko in range(KO):
                        nc.tensor.matmul(
                            ph[:, :tn],
                            w1_sb[:, e, ko, mf * P : (mf + 1) * P],
                            x_sb[:, ko, :tn],
                            start=(ko == 0),
                            stop=(ko == KO - 1),
                        )
                    nc.scalar.activation(
                        h_sb[:, mf, :tn],
                        ph[:, :tn],
                        mybir.ActivationFunctionType.Silu,
                    )

                for mt in range(tn // P):
                    mtg = t0 // P + mt
                    y_sb = ypool.tile([P, D_MODEL], dt.bfloat16, tag="y", name="y_sb")
                    for nf in range(D_MODEL // 512):
                        py = ps2.tile([P, 512], dt.float32, tag="py", name="py")
                        for mf in range(MS):
                            nc.tensor.matmul(
                                py[:],
                                h_sb[:, mf, mt * P : (mt + 1) * P],
                                w2_sb[:, e, mf, nf * 512 : (nf + 1) * 512],
                                start=(mf == 0),
                                stop=(mf == MS - 1),
                            )
                        nc.vector.tensor_scalar_mul(
                            y_sb[:, nf * 512 : (nf + 1) * 512],
                            py[:],
                            g_sb[:, mtg : mtg + 1],
                        )
                    nc.scalar.dma_start(y.ap()[:, mtg, :], y_sb[:])

    nc.compile()
    return nc


def _route(xf: np.ndarray, router: np.ndarray):
    """Host-side top-2 routing. Groups (token, slot) pairs by expert, pads
    each expert segment to a multiple of 128."""
    T = xf.shape[0]
    logits = xf @ router  # [T, E] f32
    # top-2 (desc value, ties -> lower index, matching jax.lax.top_k)
    ti = np.argsort(-logits, axis=1, kind="stable")[:, :TOP_K]  # [T, 2]
    tv = np.take_along_axis(logits, ti, axis=1)
    ex = np.exp(tv - tv[:, 0:1])
    w = (ex / ex.sum(axis=1, keepdims=True)).astype(np.float32)  # [T, 2]

    experts_all = ti.T.ravel()  # [2T] slot-major
    gates_all = w.T.ravel()
    tokens_all = np.tile(np.arange(T, dtype=np.int64), TOP_K)

    order = np.argsort(experts_all, kind="stable")
    sorted_experts = experts_all[order]
    counts = np.bincount(sorted_experts, minlength=N_EXPERTS)
    seg_sizes = tuple(int(math.ceil(c / P)) * P if c else 0 for c in counts)
    starts_pad = np.concatenate([[0], np.cumsum(seg_sizes)[:-1]])
    starts = np.concatenate([[0], np.cumsum(counts)[:-1]])
    rank = np.arange(TOP_K * T) - starts[sorted_experts]
    R = int(sum(seg_sizes))

    slot_of_pair = np.empty(TOP_K * T, dtype=np.int64)
    slot_of_pair[order] = starts_pad[sorted_experts] + rank

    slot_token = np.full(R, T, dtype=np.int64)  # T = zero-column sentinel
    slot_token[slot_of_pair] = tokens_all
    slot_gate = np.zeros(R, dtype=np.float32)
    slot_gate[slot_of_pair] = gates_all
    return seg_sizes, slot_token, slot_gate, slot_of_pair


def kernel(x, router, W1, W2, _trace=False):
    from concourse.bass_utils import run_bass_kernel_spmd

    B, S, d = x.shape
    T = B * S
    xf = np.ascontiguousarray(x.reshape(T, d), dtype=np.float32)

    seg_sizes, slot_token, slot_gate, slot_of_pair = _route(xf, np.asarray(router))
    R = int(sum(seg_sizes))
    NT = R // P

    bf16 = ml_dtypes.bfloat16
    # x^T with a trailing zero column as padding sentinel, partition-major:
    # [128, KO, T+1]
    xfT = np.concatenate([xf.T, np.zeros((d, 1), np.float32)], axis=1).astype(bf16)
    xfT = np.ascontiguousarray(xfT.reshape(KO, P, T + 1).transpose(1, 0, 2))
    x_dev = np.ascontiguousarray(xfT[:, :, slot_token])  # [128, KO, R]

    W1b = np.asarray(W1).astype(bf16)  # [E, 1024, 4096]
    W2b = np.asarray(W2).astype(bf16)  # [E, 4096, 1024]
    g_dev = np.ascontiguousarray(slot_gate.reshape(NT, P).T)  # [128, NT]

    key = seg_sizes
    nc = _KERNEL_CACHE.get(key)
    if nc is None:
        nc = _build_device_kernel(seg_sizes)
        _KERNEL_CACHE[key] = nc

    in_maps = []
    for c in range(N_CORES):
        sl = slice(c * DS, (c + 1) * DS)
        # [128, E, KO, DS]
        w1_c = np.ascontiguousarray(
            W1b[:, :, sl].reshape(N_EXPERTS, KO, P, DS).transpose(2, 0, 1, 3)
        )
        # [128, E, MS, D_MODEL]
        w2_c = np.ascontiguousarray(
            W2b[:, sl, :].reshape(N_EXPERTS, MS, P, D_MODEL).transpose(2, 0, 1, 3)
        )
        in_maps.append({"x": x_dev, "w1": w1_c, "w2": w2_c, "g": g_dev})

    kw = {}
    if _trace:
        kw = {"trace": True, "trace_cores": list(range(N_CORES))}
    res = run_bass_kernel_spmd(nc, in_maps, core_ids=list(range(N_CORES)), **kw)

    # sum the 8 partial outputs in f32; y layout [128, NT, 1024] -> [R, 1024]
    acc = np.zeros((R, D_MODEL), np.float32)
    for c in range(N_CORES):
        yc = res.results[c]["y"]  # [128, NT, 1024] bf16
        acc += yc.transpose(1, 0, 2).reshape(R, D_MODEL).astype(np.float32)

    out = acc[slot_of_pair[:T]] + acc[slot_of_pair[T:]]
    out = out.reshape(B, S, d).astype(np.float32)
    if _trace:
        return out, res
    return out
